# revision 44
# baseline (speedup 1.0000x reference)
import ml_dtypes
import numpy as np

B, CIN, H, W = 2, 16, 64, 64
COUT, P = 64, 3
K = 3
I_TOT = CIN * K * K
N_CORES = 8
ROWS_PER_CORE = 16
N_LOC = ROWS_PER_CORE * W
KCH = K * CIN
XFREE = (ROWS_PER_CORE + 2) * W
KO = K * COUT

_STATE = {}


def _install_prof_shim():
    try:
        import sys, types

        if "antenv.axon_hooks" not in sys.modules:
            mod = types.ModuleType("antenv.axon_hooks")
            holder = [None]
            mod.set_axon_ntff_profile_hook = lambda h: holder.__setitem__(0, h)
            mod.get_axon_ntff_profile_hook = lambda: holder[0]
            sys.modules["antenv.axon_hooks"] = mod
            import antenv

            antenv.axon_hooks = mod
            try:
                from trn_agent_boot.trn_boot import _ntff_profile_via_ctypes

                hook = _ntff_profile_via_ctypes("/opt/axon/libaxon_pjrt.so")
                mod.set_axon_ntff_profile_hook(hook)
            except Exception:
                pass
        import concourse.bass_utils as bu

        if getattr(bu.upload_artifacts, "__name__", "") != "<lambda>":
            bu.upload_artifacts = lambda tmpdir: tmpdir
    except Exception:
        pass


def _build_program(width):
    import concourse.bass as bass
    import concourse.mybir as mybir
    from concourse import bacc

    f32 = mybir.dt.float32
    bf16 = mybir.dt.bfloat16
    sub = mybir.AluOpType.subtract
    mult = mybir.AluOpType.mult
    add_op = mybir.AluOpType.add
    act_id = mybir.ActivationFunctionType.Identity
    act_relu = mybir.ActivationFunctionType.Relu
    inv_w = float(1.0 / width)

    nc = bacc.Bacc(
        "TRN2", target_bir_lowering=False, num_devices=N_CORES,
        enable_partition_id=False,
    )
    x_d = nc.dram_tensor("x3b", [KCH, XFREE], bf16, kind="ExternalInput")
    tbl_d = nc.dram_tensor("tbl3", [KCH, 2 * K * KO], f32, kind="ExternalInput")
    out_d = nc.dram_tensor("out", [128, 512], bf16, kind="ExternalOutput")

    from contextlib import ExitStack

    with ExitStack() as ctx:
        e = ctx.enter_context
        xbf = e(nc.sbuf_tensor([KCH, XFREE], bf16))
        tblv = e(nc.sbuf_tensor([KCH, K * KO], f32))
        tblp = e(nc.sbuf_tensor([KCH, K * KO], f32))
        negp1 = e(nc.sbuf_tensor([KCH, 1], f32))
        pos3 = e(nc.sbuf_tensor([KCH, XFREE], bf16))
        a0f = e(nc.sbuf_tensor([KCH, KO], f32))
        a2bf = e(nc.sbuf_tensor([KCH, KO], bf16))
        w1_all = e(nc.sbuf_tensor([KCH, KO], bf16))
        wd_all = e(nc.sbuf_tensor([KCH, KO], bf16))
        tmp = e(nc.sbuf_tensor([KCH, KO], f32))
        b1 = e(nc.sbuf_tensor([KCH, KO], bf16))
        ones = e(nc.sbuf_tensor([KCH, 1], bf16))
        bias = e(nc.sbuf_tensor([128, 1], f32))
        ob = e(nc.sbuf_tensor([128, 512], bf16))
        psA = e(nc.psum_tensor([128, 512], f32))
        psb = e(nc.psum_tensor([128, 1], f32))
        s_in = e(nc.semaphore("s_in"))
        s_x = e(nc.semaphore("s_x"))
        s_v2 = e(nc.semaphore("s_v2"))
        s_tp = e(nc.semaphore("s_tp"))
        s_np = e(nc.semaphore("s_np"))
        s_w = e(nc.semaphore("s_w"))
        s_p3 = e(nc.semaphore("s_p3"))
        s_b1 = e(nc.semaphore("s_b1"))
        s_mm = e(nc.semaphore("s_mm"))
        s_mmn = e(nc.semaphore("s_mmn"))
        s_bc = e(nc.semaphore("s_bc"))
        s_ev = e(nc.semaphore("s_ev"))
        s_out = e(nc.semaphore("s_out"))
        block = e(nc.Block())

        posv = tblp.ap().rearrange("p (k x) -> p k x", k=K)
        valv = tblv.ap().rearrange("p (k x) -> p k x", k=K)

        @block.sync
        def _(sync):
            sync.dma_start(
                out=tblv[:, 0 : 2 * KO], in_=tbl_d.ap()[:, K * KO : K * KO + 2 * KO]
            ).then_inc(s_in, 16)
            sync.dma_start(out=xbf[:], in_=x_d.ap()[:]).then_inc(s_x, 16)
            sync.dma_start(
                out=tblv[:, 2 * KO :], in_=tbl_d.ap()[:, K * KO + 2 * KO :]
            ).then_inc(s_v2, 16)
            sync.wait_ge(s_ev, 1)
            sync.dma_start(out=out_d.ap()[:], in_=ob[:]).then_inc(s_out, 16)
            sync.wait_ge(s_out, 16)

        @block.scalar
        def _(scalar):
            scalar.dma_start(out=tblp[:], in_=tbl_d.ap()[:, 0 : K * KO]).then_inc(
                s_tp, 16
            )
            scalar.wait_ge(s_x, 16)
            scalar.wait_ge(s_np, 1)
            nc.scalar.activation(pos3[:], xbf[:], act_relu, bias=negp1[:]).then_inc(
                s_p3, 1
            )
            scalar.wait_ge(s_mmn, 1)
            scalar.wait_ge(s_bc, 1)
            nc.scalar.activation(
                ob[:], psA.ap()[:], act_id, bias=bias[:], scale=inv_w
            ).then_inc(s_ev, 1)

        @block.vector
        def _(vector):
            vector.wait_ge(s_tp, 16)
            nc.vector.tensor_scalar_mul(negp1[:], tblp[:, KO : KO + 1], -1.0).then_inc(
                s_np, 1
            )
            vector.wait_ge(s_in, 16)
            nc.vector.tensor_tensor(
                w1_all[:], valv[:, 1, :], valv[:, 0, :], sub
            ).then_inc(s_w, 1)
            vector.wait_ge(s_v2, 16)
            nc.vector.tensor_tensor(a2bf[:], valv[:, 2, :], valv[:, 1, :], sub)
            nc.vector.tensor_tensor(wd_all[:], a2bf[:], w1_all[:], sub).then_inc(
                s_w, 1
            )
            nc.vector.tensor_tensor(a0f[:], valv[:, 1, :], valv[:, 0, :], sub)
            nc.vector.tensor_tensor(tmp[:], posv[:, 1, :], a0f[:], mult)
            nc.vector.tensor_scalar_mul(tmp[:], tmp[:], inv_w)
            nc.vector.tensor_tensor(b1[:], valv[:, 1, :], tmp[:], sub)
            nc.vector.memset(ones[:], 1.0).then_inc(s_b1, 1)
            vector.wait_ge(s_mm, 1)
            nc.vector.tensor_copy(bias[:], psb.ap()[:]).then_inc(s_bc, 1)

        @block.tensor
        def _(tensor):
            w1g = [w1_all.ap()[:, g * COUT : (g + 1) * COUT] for g in range(K)]
            wdg = [wd_all.ap()[:, g * COUT : (g + 1) * COUT] for g in range(K)]

            def mm_pair(wt, rhs_t, kh, start, stop):
                last = None
                for cg in (0, COUT):
                    base = kh * W + (cg // COUT) * 512
                    last = nc.tensor.matmul(
                        psA.ap()[cg : cg + COUT, :],
                        wt,
                        rhs_t[:, base : base + 512],
                        start=start,
                        stop=stop,
                        tile_position=(0, cg),
                        skip_group_check=True,
                    )
                return last

            tensor.wait_ge(s_x, 16)
            tensor.wait_ge(s_w, 1)
            mm_pair(w1g[0], xbf.ap(), 0, True, False)
            mm_pair(w1g[1], xbf.ap(), 1, False, False)
            mm_pair(w1g[2], xbf.ap(), 2, False, False)
            tensor.wait_ge(s_w, 2)
            tensor.wait_ge(s_p3, 1)
            mm_pair(wdg[0], pos3.ap(), 0, False, False)
            mm_pair(wdg[1], pos3.ap(), 1, False, False)
            tensor.wait_ge(s_b1, 1)
            for cg in (0, COUT):
                for g in range(K):
                    ins = nc.tensor.matmul(
                        psb.ap()[cg : cg + COUT, :],
                        b1[:, g * COUT : (g + 1) * COUT],
                        ones[:],
                        start=(g == 0),
                        stop=(g == K - 1),
                        tile_position=(0, cg),
                        skip_group_check=True,
                    )
            ins.then_inc(s_mm, 1)
            mm_pair(wdg[2], pos3.ap(), 2, False, True).then_inc(s_mmn, 1)

    nc.compile()
    return nc


def _fast_path_ok(positions):
    if positions.shape != (I_TOT, COUT, P):
        return False
    p = positions
    if np.ptp(p[:, :, 1]) != 0.0:
        return False
    w01 = p[:, :, 1] - p[:, :, 0]
    w12 = p[:, :, 2] - p[:, :, 1]
    w = w01.flat[0]
    if w <= 0.0 or np.ptp(w01) != 0.0 or np.any(w12 != w):
        return False
    return True


def _reference_numpy(x, positions, values):
    xf = x.astype(np.float32)
    Bs, C, Hs, Ws = xf.shape
    xp = np.pad(xf, ((0, 0), (0, 0), (1, 1), (1, 1)))
    cols = [xp[:, :, i : i + Hs, j : j + Ws] for i in range(K) for j in range(K)]
    pch = np.stack(cols, 2).reshape(Bs, C * K * K, Hs * Ws)
    X = pch.transpose(0, 2, 1).reshape(-1, C * K * K)
    Np, Ii = X.shape
    Pp = positions.shape[-1]
    out = np.zeros((Np, positions.shape[1]), np.float32)
    chunk = 1024
    for st in range(0, Np, chunk):
        xb = X[st : st + chunk, :, None]
        idx = np.sum(xb[..., None] >= positions[None], axis=-1)
        idx = np.clip(idx, 1, Pp - 1)
        f = np.zeros((xb.shape[0], Ii, positions.shape[1]), np.float32)
        for s in range(1, Pp):
            x0 = positions[:, :, s - 1]
            x1 = positions[:, :, s]
            y0 = values[:, :, s - 1]
            y1 = values[:, :, s]
            t = (xb - x0) / (x1 - x0)
            f = np.where(idx == s, y0 + t * (y1 - y0), f)
        out[st : st + chunk] = f.sum(axis=1)
    O = out.shape[-1]
    return out.reshape(Bs, Hs * Ws, O).transpose(0, 2, 1).reshape(Bs, O, Hs, Ws)


def kernel(x, positions, values):
    x = np.ascontiguousarray(x, dtype=np.float32)
    positions = np.ascontiguousarray(positions, dtype=np.float32)
    values = np.ascontiguousarray(values, dtype=np.float32)

    if not _fast_path_ok(positions):
        return _reference_numpy(x, positions, values)

    _install_prof_shim()
    from concourse.bass_utils import run_bass_kernel_spmd

    width = float(positions[0, 0, 1] - positions[0, 0, 0])
    key = ("nc", width)
    if key not in _STATE:
        _STATE[key] = _build_program(width)
    nc = _STATE[key]

    pos5 = positions.reshape(CIN, K, K, COUT, P).transpose(0, 2, 4, 1, 3)
    val5 = values.reshape(CIN, K, K, COUT, P).transpose(0, 2, 4, 1, 3)
    tbl = np.ascontiguousarray(
        np.concatenate(
            [pos5.reshape(KCH, K * K * COUT), val5.reshape(KCH, K * K * COUT)], axis=1
        )
    )

    xp = np.pad(x, ((0, 0), (0, 0), (1, 1), (1, 1)))
    in_maps = []
    for k in range(N_CORES):
        b, y0 = divmod(k, N_CORES // B)
        y0 *= ROWS_PER_CORE
        slab = xp[b, :, y0 : y0 + ROWS_PER_CORE + 2, :]
        x3 = np.empty((CIN, K, ROWS_PER_CORE + 2, W), np.float32)
        for kw in range(K):
            x3[:, kw] = slab[:, :, kw : kw + W]
        in_maps.append(
            {"x3b": x3.reshape(KCH, XFREE).astype(ml_dtypes.bfloat16), "tbl3": tbl}
        )

    res = run_bass_kernel_spmd(nc, in_maps, core_ids=list(range(N_CORES)))
    _STATE["last_result"] = res

    out = np.empty((B, COUT, H, W), np.float32)
    for k in range(N_CORES):
        b, y0 = divmod(k, N_CORES // B)
        y0 *= ROWS_PER_CORE
        o2 = (
            res.results[k]["out"].astype(np.float32).reshape(2, COUT, 512)
            .transpose(1, 0, 2)
        )
        out[b, :, y0 : y0 + ROWS_PER_CORE, :] = o2.reshape(COUT, ROWS_PER_CORE, W)
    return out
